# revision 37
# baseline (speedup 1.0000x reference)
"""Trainium2 Bass kernel for blocksparse (sink+local) Llama attention.

Sharding: tensor-parallel by head across 8 NeuronCores. Core c computes
q-heads [4c, 4c+4) and kv-head c (the matching GQA group):
  - q/k/v projections column-parallel (per-core weight slices)
  - RoPE + blocksparse streaming attention fully head-local
  - o_proj row-parallel: each core emits a partial [S, HID] product
The row-parallel all-reduce is done at unshard time on the host (an 8-way
fp32 sum), which is far cheaper than an on-device collective here.

Attention computes S^T = K_blk^T Q_blk per granted block with the 4 GQA
heads batched side-by-side in the moving operand (N=512 matmuls — the PE
sequencer is the co-bottleneck at N=128): exp(S^T) lands in SBUF as P^T
tiles for the P^T @ V accumulation, softmax column sums come from a DVE
free-axis reduce over the per-block tiles plus a single all-ones stationary
matmul (which broadcasts them across partitions for free), and the 1/sum
normalization is deferred one block so the PE never waits on the softmax
chain. V is projected directly in the natural [s, d] PV layout.

Everything on device runs in bf16 with fp32 PSUM accumulation.
"""

import sys

sys.path.insert(0, "/opt/trn_rl_repo")

import math
from contextlib import ExitStack

import ml_dtypes
import numpy as np

import concourse.bass as bass
import concourse.tile as tile
from concourse import bacc, mybir
from concourse.masks import make_lower_triangular

BF16 = mybir.dt.bfloat16
F32 = mybir.dt.float32
NPBF = ml_dtypes.bfloat16

N_CORES = 8
S = 4096
HID = 4096
NH, NKV, D = 32, 8, 128
QH = NH // N_CORES          # 4 q heads per core
BLK = 128
NB = S // BLK               # 32 blocks
LOCAL_NB = 8
SCHUNK = 512                # s-columns processed per phase-1 step
NSC = S // SCHUNK           # 8
HT = HID // 128             # 32 contraction tiles
MASK_VAL = -30000.0
THETA = 10000.0


def _rope_into(nc, pool, dst, ps, cos_c, sin_c, width):
    """dst(bf16) = ps * cos_c + swap_halves(ps) * sin_c  (sin_c sign-baked).

    ps is a [128, width] fp32 PSUM tile holding a projection output d-block;
    partition p is feature dim d. swap_halves pairs d <-> d+64.
    dst may be a 3-D AP; t0/t1 are viewed with the same shape for the add.
    """
    t0 = pool.tile([128, SCHUNK], F32, tag="rope_t0", name="rope_t0")
    t1 = pool.tile([128, SCHUNK], F32, tag="rope_t1", name="rope_t1")
    nc.vector.tensor_mul(t0[:, :width], ps[:, :width], cos_c[:, :width])
    nc.vector.tensor_mul(t1[0:64, :width], ps[64:128, :width], sin_c[0:64, :width])
    nc.vector.tensor_mul(t1[64:128, :width], ps[0:64, :width], sin_c[64:128, :width])
    if len(dst.shape) == 3:
        b = dst.shape[1]
        qw = dst.shape[2]
        nc.vector.tensor_add(
            dst,
            t0[:, :width].rearrange("p (b q) -> p b q", b=b, q=qw),
            t1[:, :width].rearrange("p (b q) -> p b q", b=b, q=qw),
        )
    else:
        nc.vector.tensor_add(dst, t0[:, :width], t1[:, :width])


def _emit_body(nc, tc, persist, aps):
    hsT, wq, wk, wv, wo, cos2, sin2, out_p = aps

    # Head-batched layouts: block i occupies cols [i*512, (i+1)*512), with the
    # 4 q-heads side by side (head hq at [i*512 + hq*128, ...)). This makes the
    # attention matmuls N=512 (one per k-block, covering all 4 GQA heads that
    # share this core's kv head) instead of N=128 per (head, block) — 4x fewer
    # PE instructions, which matters because the PE NX sequencer is the
    # co-bottleneck at N=128.
    qT4 = persist.tile([128, NB * 512], BF16, name="qT4")     # [d | (blk, hq, q)]
    kT = persist.tile([128, S], BF16, name="kT")              # [d | s]
    vN = persist.tile([128, NB * 128], BF16, name="vN")       # [s_in_blk | (blk, d)]
    attnT4 = persist.tile([128, NB * 512], BF16, name="attnT4")  # like qT4
    wq_sb = persist.tile([128, HT * QH * 128], BF16, name="wq_sb")
    wk_sb = persist.tile([128, HT * 128], BF16, name="wk_sb")
    wv_sb = persist.tile([128, HT * 128], BF16, name="wv_sb")
    wo_sb = persist.tile([128, QH * HID], BF16, name="wo_sb")
    ones = persist.tile([128, 128], BF16, name="ones")
    tri4 = persist.tile([128, 512], F32, name="tri4")   # causal mask x4 heads

    nc.vector.memset(ones, 1.0)
    make_lower_triangular(nc, tri4[:, 0:128], val=MASK_VAL, diag=False)
    for r in range(1, 4):
        nc.vector.tensor_copy(tri4[:, r * 128 : (r + 1) * 128], tri4[:, 0:128])

    GRP = 8                      # htiles per grouped DMA
    WGRP = GRP * 128

    def _load_w_group(g):
        # one 3D-AP DMA per weight for 8 htiles: [ht*128+p, c] -> [p, ht*C+c]
        nc.sync.dma_start(
            out=wq_sb[:, g * GRP * 512 : (g + 1) * GRP * 512].rearrange(
                "p (t c) -> p t c", c=512
            ),
            in_=wq[g * WGRP : (g + 1) * WGRP, :].rearrange("(t p) c -> p t c", p=128),
        )
        nc.sync.dma_start(
            out=wk_sb[:, g * GRP * 128 : (g + 1) * GRP * 128].rearrange(
                "p (t c) -> p t c", c=128
            ),
            in_=wk[g * WGRP : (g + 1) * WGRP, :].rearrange("(t p) c -> p t c", p=128),
        )
        nc.sync.dma_start(
            out=wv_sb[:, g * GRP * 128 : (g + 1) * GRP * 128].rearrange(
                "p (t c) -> p t c", c=128
            ),
            in_=wv[g * WGRP : (g + 1) * WGRP, :].rearrange("(t p) c -> p t c", p=128),
        )

    # ---------------- phase 1: projections + RoPE ----------------
    ph1 = ExitStack()
    hs_pool = ph1.enter_context(tc.tile_pool(name="hs_pool", bufs=4))
    tab_pool = ph1.enter_context(tc.tile_pool(name="tab_pool", bufs=1))
    rope_pool = ph1.enter_context(tc.tile_pool(name="rope_pool", bufs=1))
    ps_p = ph1.enter_context(tc.tile_pool(name="ps_p", bufs=6, space="PSUM"))
    ps_v = ph1.enter_context(tc.tile_pool(name="ps_v", bufs=2, space="PSUM"))

    def _q_dst(sc, db):
        # strided view of qT4 for this chunk+head: [p, 4 blocks, 128 q]
        return qT4[:, sc * 2048 : (sc + 1) * 2048].rearrange(
            "p (b q) -> p b q", q=512
        )[:, :, db * 128 : (db + 1) * 128]

    def _v_pass(sc, _hs):
        # v projection in natural [s, d] layout (stationary = hs slice):
        # the PV matmul needs s on partitions, and this form is proven to
        # run at model rate on hardware.
        for sb in range(SCHUNK // 128):
            g = sc * (SCHUNK // 128) + sb
            psv = ps_v.tile([128, 128], F32, tag="ps_v", name="ps_vp")
            for ht in range(HT):
                nc.tensor.matmul(
                    psv,
                    lhsT=_hs(ht)[:, sb * 128 : (sb + 1) * 128],
                    rhs=wv_sb[:, ht * 128 : (ht + 1) * 128],
                    start=(ht == 0),
                    stop=(ht == HT - 1),
                )
            nc.scalar.copy(out=vN[:, g * 128 : (g + 1) * 128], in_=psv)

    for sc in range(NSC):
        scol = slice(sc * SCHUNK, (sc + 1) * SCHUNK)
        cos_c = tab_pool.tile([128, SCHUNK], F32, tag="cos_c", name="cos_c")
        sin_c = tab_pool.tile([128, SCHUNK], F32, tag="sin_c", name="sin_c")
        hs_grps = []
        for g in range(HT // GRP):
            if sc == 0:
                # interleave weight group loads with the first hs chunk so
                # the first matmuls aren't stuck behind the weight prefetch
                _load_w_group(g)
            hg = hs_pool.tile([128, GRP * SCHUNK], BF16, tag="hs",
                              name=f"hs_{sc}_{g}")
            nc.sync.dma_start(
                out=hg.rearrange("p (t c) -> p t c", c=SCHUNK),
                in_=hsT[g * WGRP : (g + 1) * WGRP, scol].rearrange(
                    "(t p) c -> p t c", p=128
                ),
            )
            hs_grps.append(hg)

        def _hs(ht):
            return hs_grps[ht // GRP][:, (ht % GRP) * SCHUNK : (ht % GRP + 1) * SCHUNK]
        # tables are consumed by RoPE at chunk end; keep them out of the
        # DMA queue's critical head during the compute-feeding loads
        nc.sync.dma_start(out=cos_c, in_=cos2[:, scol])
        nc.sync.dma_start(out=sin_c, in_=sin2[:, scol])
        if sc == 0:
            # wo is only read in phase 2; load it behind everything else
            nc.sync.dma_start(
                out=wo_sb.rearrange("p (t c) -> p t c", c=HID),
                in_=wo.rearrange("(t p) c -> p t c", p=128),
            )
        # q (4 head d-blocks) + k + v projections, all in transposed layout
        # [d, s] with N=512 matmuls. For the first chunk go ht-outer with all
        # 6 psums live so PE has ~1.3us of work per arriving hs tile
        # (DMA-matched at startup); later chunks are fully prefetched and use
        # the output-outer order (1 psum bank at a time).
        if sc == 0:
            ps5 = [
                ps_p.tile([128, SCHUNK], F32, tag="pp", name=f"ps5_{r}")
                for r in range(5)
            ]
            for ht in range(HT):
                for db in range(QH):
                    nc.tensor.matmul(
                        ps5[db],
                        lhsT=wq_sb[:, ht * 512 + db * 128 : ht * 512 + (db + 1) * 128],
                        rhs=_hs(ht),
                        start=(ht == 0),
                        stop=(ht == HT - 1),
                    )
                nc.tensor.matmul(
                    ps5[4],
                    lhsT=wk_sb[:, ht * 128 : (ht + 1) * 128],
                    rhs=_hs(ht),
                    start=(ht == 0),
                    stop=(ht == HT - 1),
                )
            for db in range(QH):
                _rope_into(nc, rope_pool, _q_dst(sc, db), ps5[db],
                           cos_c, sin_c, SCHUNK)
            _rope_into(nc, rope_pool, kT[:, scol], ps5[4], cos_c, sin_c, SCHUNK)
            _v_pass(sc, _hs)
        else:
            # v first, then q, then k last — so the hs groups' last reader is
            # the k-pass's early htiles, releasing buffers for the next
            # chunk's prefetch as early as possible.
            _v_pass(sc, _hs)
            for db in range(QH):
                ps = ps_p.tile([128, SCHUNK], F32, tag="pp", name="ps_qp")
                for ht in range(HT):
                    nc.tensor.matmul(
                        ps,
                        lhsT=wq_sb[:, ht * 512 + db * 128 : ht * 512 + (db + 1) * 128],
                        rhs=_hs(ht),
                        start=(ht == 0),
                        stop=(ht == HT - 1),
                    )
                _rope_into(nc, rope_pool, _q_dst(sc, db), ps,
                           cos_c, sin_c, SCHUNK)
            psk = ps_p.tile([128, SCHUNK], F32, tag="pp", name="ps_kp")
            for ht in range(HT):
                nc.tensor.matmul(
                    psk,
                    lhsT=wk_sb[:, ht * 128 : (ht + 1) * 128],
                    rhs=_hs(ht),
                    start=(ht == 0),
                    stop=(ht == HT - 1),
                )
            _rope_into(nc, rope_pool, kT[:, scol], psk, cos_c, sin_c, SCHUNK)
    ph1.close()

    # -------- phase 2+3 merged: attention with o_proj software-pipelined --------
    # Per block i (all 4 heads at once, N=512):
    #   S^T_j = kT_j^T @ qT4_i per granted block j  ->  exp on ACT  ->
    #   PV accumulates over j; softmax sums via a DVE free-axis reduce over the
    #   j tiles + ONE all-ones matmul (partition-broadcast); normalize is
    #   deferred one iteration, o_proj runs two blocks behind as PE filler
    #   while ACT works through the exp chain.
    ph2 = ExitStack()
    pt_pool = ph2.enter_context(tc.tile_pool(name="pt_pool", bufs=3))
    red_pool = ph2.enter_context(tc.tile_pool(name="red_pool", bufs=2))
    rb_pool = ph2.enter_context(tc.tile_pool(name="rb_pool", bufs=2))
    ob_pool = ph2.enter_context(tc.tile_pool(name="ob_pool", bufs=4))
    ps_S = ph2.enter_context(tc.tile_pool(name="ps_S", bufs=3, space="PSUM"))
    ps_O = ph2.enter_context(tc.tile_pool(name="ps_O", bufs=2, space="PSUM"))
    ps_sum = ph2.enter_context(tc.tile_pool(name="ps_sum", bufs=1, space="PSUM"))
    ps_o3 = ph2.enter_context(tc.tile_pool(name="ps_o3", bufs=2, space="PSUM"))

    def _og(sb, ct):
        # o_proj partial for s-block sb, output cols [ct*512, (ct+1)*512)
        ps = ps_o3.tile([128, 512], F32, tag="o3", name="o3")
        for ht in range(QH):
            nc.tensor.matmul(
                ps,
                lhsT=attnT4[:, (sb * 4 + ht) * 128 : (sb * 4 + ht + 1) * 128],
                rhs=wo_sb[:, ht * HID + ct * 512 : ht * HID + (ct + 1) * 512],
                start=(ht == 0),
                stop=(ht == QH - 1),
            )
        ob = ob_pool.tile([128, 512], BF16, tag="ob", name="ob")
        # split the PSUM->SBUF drain across ACT and DVE so neither engine's
        # stream becomes co-critical with the exp chain
        if ct % 2 == 0:
            nc.scalar.copy(out=ob, in_=ps)
        else:
            nc.vector.tensor_copy(ob, ps)
        nc.sync.dma_start(
            out=out_p[sb * 128 : (sb + 1) * 128, ct * 512 : (ct + 1) * 512],
            in_=ob,
        )

    def _finish(pend):
        # ones-matmul broadcast of the j-reduced sums, then 1/sum and the
        # deferred normalize of block ip's attention output.
        red_bf, O_prev, ip = pend
        sum_ps = ps_sum.tile([128, 512], F32, tag="sum", name="sum_ps")
        nc.tensor.matmul(sum_ps, lhsT=ones, rhs=red_bf, start=True, stop=True)
        rb = rb_pool.tile([128, 512], F32, tag="rb", name="rb")
        nc.vector.reciprocal_approx_fast(rb, sum_ps)
        nc.vector.tensor_mul(
            attnT4[:, ip * 512 : (ip + 1) * 512], O_prev, rb
        )

    pending = None
    for i in range(NB):
        L = min(i, LOCAL_NB)       # number of local blocks
        js = i - L + 1             # first local block (>= 1 when L > 0)
        blocks = list(range(js, i + 1)) if i >= 1 else []
        blocks.append(0)           # sink block last
        nblk = len(blocks)
        diag_bi = L - 1 if i >= 1 else 0
        qs = qT4[:, i * 512 : (i + 1) * 512]
        PT_all = pt_pool.tile([128, 9 * 512], BF16, tag="PT", name="PT_all")
        acc = red_pool.tile([128, 512], F32, tag="acc", name="acc") \
            if nblk > 1 else None

        def _st(bi, j):
            sp = ps_S.tile([128, 512], F32, tag="S", name="S_ps")
            nc.tensor.matmul(sp, lhsT=kT[:, j * 128 : (j + 1) * 128], rhs=qs,
                             start=True, stop=True)
            if bi == diag_bi:
                nc.vector.tensor_add(sp, sp, tri4)
            nc.scalar.activation(
                out=PT_all[:, bi * 512 : (bi + 1) * 512], in_=sp,
                func=mybir.ActivationFunctionType.Exp,
            )
            # fold the fresh exp tile into the f32 softmax-sum accumulator
            # right away: the adds ride along behind ACT's exp stream, so the
            # total is ready ~one add after the last exp
            if bi == 1:
                nc.vector.tensor_add(acc, PT_all[:, 0:512], PT_all[:, 512:1024])
            elif bi >= 2:
                nc.vector.tensor_add(
                    acc, acc, PT_all[:, bi * 512 : (bi + 1) * 512]
                )

        for bi in range(min(3, nblk)):
            _st(bi, blocks[bi])
        if pending is not None:
            _finish(pending)
        # interleave the remaining S^T with o_proj filler for block i-2
        ogs = list(range(8)) if i >= 2 else []
        for bi in range(3, nblk):
            if ogs:
                _og(i - 2, ogs.pop(0))
            _st(bi, blocks[bi])
        for ct in ogs:
            _og(i - 2, ct)
        # P^T @ V accumulated over granted blocks
        O_ps = ps_O.tile([128, 512], F32, tag="O", name="O_ps")
        for bi, j in enumerate(blocks):
            nc.tensor.matmul(
                O_ps, lhsT=vN[:, j * 128 : (j + 1) * 128],
                rhs=PT_all[:, bi * 512 : (bi + 1) * 512],
                start=(bi == 0), stop=(bi == nblk - 1),
            )
        # softmax sums: cast the f32 accumulator for the all-ones matmul
        if nblk > 1:
            red_bf = red_pool.tile([128, 512], BF16, tag="redb", name="redb")
            nc.scalar.copy(out=red_bf, in_=acc)
        else:
            red_bf = PT_all[:, 0:512]
        pending = (red_bf, O_ps, i)

    _finish(pending)
    for sb in (NB - 2, NB - 1):
        for ct in range(8):
            _og(sb, ct)
    ph2.close()


def build_kernel(nc, reps=1):
    hsT = nc.dram_tensor("hsT", [HID, S], BF16, kind="ExternalInput").ap()
    wq = nc.dram_tensor("wq", [HID, QH * D], BF16, kind="ExternalInput").ap()
    wk = nc.dram_tensor("wk", [HID, D], BF16, kind="ExternalInput").ap()
    wv = nc.dram_tensor("wv", [HID, D], BF16, kind="ExternalInput").ap()
    wo = nc.dram_tensor("wo", [QH * D, HID], BF16, kind="ExternalInput").ap()
    cos2 = nc.dram_tensor("cos2", [128, S], F32, kind="ExternalInput").ap()
    sin2 = nc.dram_tensor("sin2", [128, S], F32, kind="ExternalInput").ap()
    out_p = nc.dram_tensor("out_p", [S, HID], BF16, kind="ExternalOutput").ap()
    aps = (hsT, wq, wk, wv, wo, cos2, sin2, out_p)

    with tile.TileContext(nc) as tc:
        with tc.tile_pool(name="persist", bufs=1) as persist:
            for _rep in range(reps):
                _emit_body(nc, tc, persist, aps)
    return nc


_NC = {}


def _get_nc(reps=1):
    if reps not in _NC:
        nc = bacc.Bacc(
            "TRN2", target_bir_lowering=False, debug=False, num_devices=N_CORES
        )
        build_kernel(nc, reps=reps)
        nc.compile()
        _NC[reps] = nc
    return _NC[reps]


def make_exec_fn(nc, n_cores=N_CORES):
    """Build a reusable sharded executor for a compiled Bass module.

    Mirrors bass2jax.run_bass_via_pjrt's multi-core branch, but without
    donation so the zero output buffers can stay device-resident across
    repeated calls (for benchmarking).
    """
    import jax
    from jax.sharding import Mesh, NamedSharding, PartitionSpec
    from jax.experimental.shard_map import shard_map

    from concourse import bass2jax

    bass2jax.install_neuronx_cc_hook()

    partition_name = nc.partition_id_tensor.name if nc.partition_id_tensor else None
    in_names, out_names, out_avals, zero_outs = [], [], [], []
    for alloc in nc.m.functions[0].allocations:
        if not isinstance(alloc, mybir.MemoryLocationSet):
            continue
        name = alloc.memorylocations[0].name
        if alloc.kind == "ExternalInput":
            if name != partition_name:
                in_names.append(name)
        elif alloc.kind == "ExternalOutput":
            out_names.append(name)
            shape = tuple(alloc.tensor_shape)
            dtype = mybir.dt.np(alloc.dtype)
            out_avals.append(jax.core.ShapedArray(shape, dtype))
            zero_outs.append(np.zeros(shape, dtype))
    all_in_names = list(in_names) + list(out_names)
    if partition_name is not None:
        all_in_names.append(partition_name)
    all_in_names = tuple(all_in_names)

    def _body(*args):
        operands = list(args)
        if partition_name is not None:
            operands.append(bass2jax.partition_id_tensor())
        outs = bass2jax._bass_exec_p.bind(
            *operands,
            out_avals=tuple(out_avals),
            in_names=all_in_names,
            out_names=tuple(out_names),
            lowering_input_output_aliases=(),
            sim_require_finite=True,
            sim_require_nnan=True,
            nc=nc,
        )
        return tuple(outs)

    devices = jax.devices()[:n_cores]
    mesh = Mesh(np.asarray(devices), ("core",))
    spec = PartitionSpec("core")
    in_specs = (spec,) * (len(in_names) + len(out_names))
    out_specs = (spec,) * len(out_names)
    fn = jax.jit(
        shard_map(
            _body, mesh=mesh, in_specs=in_specs, out_specs=out_specs, check_rep=False
        ),
        keep_unused=True,
    )
    return fn, in_names, out_names, zero_outs, NamedSharding(mesh, spec)


_EXEC = None


def _get_exec():
    global _EXEC
    if _EXEC is None:
        _EXEC = make_exec_fn(_get_nc())
    return _EXEC


def _concat_args(in_maps, in_names, zero_outs):
    concat_in = [
        np.concatenate([np.asarray(in_maps[c][nm]) for c in range(N_CORES)], axis=0)
        for nm in in_names
    ]
    concat_zeros = [
        np.zeros((N_CORES * z.shape[0], *z.shape[1:]), z.dtype) for z in zero_outs
    ]
    return concat_in + concat_zeros


def _host_inputs(hidden_states, wq, wk, wv, wo):
    hs = np.asarray(hidden_states, np.float32).reshape(S, HID)
    hsT = np.ascontiguousarray(hs.T).astype(NPBF)

    scale = 1.0 / math.sqrt(D)
    inv_freq = 1.0 / (THETA ** (np.arange(0, D, 2, dtype=np.float32) / D))
    t = np.arange(S, dtype=np.float32)
    freqs = np.outer(t, inv_freq)                      # [S, 64]
    cosT = np.cos(freqs).T.astype(np.float32)          # [64, S]
    sinT = np.sin(freqs).T.astype(np.float32)
    cos2 = np.ascontiguousarray(np.concatenate([cosT, cosT], 0))   # [128, S]
    sin2 = np.ascontiguousarray(np.concatenate([-sinT, sinT], 0))  # [128, S]

    wq = np.asarray(wq, np.float32) * scale
    in_maps = []
    for c in range(N_CORES):
        in_maps.append(
            {
                "hsT": hsT,
                "wq": np.ascontiguousarray(wq[:, c * 512 : (c + 1) * 512]).astype(NPBF),
                "wk": np.ascontiguousarray(
                    np.asarray(wk, np.float32)[:, c * 128 : (c + 1) * 128]
                ).astype(NPBF),
                "wv": np.ascontiguousarray(
                    np.asarray(wv, np.float32)[:, c * 128 : (c + 1) * 128]
                ).astype(NPBF),
                "wo": np.ascontiguousarray(
                    np.asarray(wo, np.float32)[c * 512 : (c + 1) * 512, :]
                ).astype(NPBF),
                "cos2": cos2,
                "sin2": sin2,
            }
        )
    return in_maps


def _reduce_out(out_concat):
    acc = (
        np.asarray(out_concat)
        .reshape(N_CORES, S, HID)
        .astype(np.float32)
        .sum(axis=0)
    )
    return np.ascontiguousarray(acc).reshape(1, S, HID)


def run(hidden_states, wq, wk, wv, wo):
    """Returns full fp32 output [1, S, HID]."""
    import jax

    fn, in_names, out_names, zero_outs, sh = _get_exec()
    in_maps = _host_inputs(hidden_states, wq, wk, wv, wo)
    args = _concat_args(in_maps, in_names, zero_outs)
    outs = jax.block_until_ready(fn(*args))
    return _reduce_out(outs[0])


def bench(hidden_states, wq, wk, wv, wo, iters=10):
    """Repeated device-resident executions; returns (out, per-iter seconds)."""
    import time

    import jax

    fn, in_names, out_names, zero_outs, sh = _get_exec()
    in_maps = _host_inputs(hidden_states, wq, wk, wv, wo)
    args = _concat_args(in_maps, in_names, zero_outs)
    dev_args = jax.block_until_ready([jax.device_put(a, sh) for a in args])
    outs = jax.block_until_ready(fn(*dev_args))  # warm-up + compile
    times = []
    for _ in range(iters):
        t0 = time.perf_counter()
        o = fn(*dev_args)
        jax.block_until_ready(o)
        times.append(time.perf_counter() - t0)
    # async-queued: submit all, block once — measures pipelined dispatch
    for n in (1, iters):
        t0 = time.perf_counter()
        os_ = [fn(*dev_args) for _ in range(n)]
        jax.block_until_ready(os_)
        times.append((time.perf_counter() - t0) / n)
    return _reduce_out(outs[0]), times


def kernel(hidden_states, wq, wk, wv, wo):
    return run(hidden_states, wq, wk, wv, wo)



# revision 38
# speedup vs baseline: 1.0271x; 1.0271x over previous
"""Trainium2 Bass kernel for blocksparse (sink+local) Llama attention.

Sharding: tensor-parallel by head across 8 NeuronCores. Core c computes
q-heads [4c, 4c+4) and kv-head c (the matching GQA group):
  - q/k/v projections column-parallel (per-core weight slices)
  - RoPE + blocksparse streaming attention fully head-local
  - o_proj row-parallel: each core emits a partial [S, HID] product
The row-parallel all-reduce is done at unshard time on the host (an 8-way
fp32 sum), which is far cheaper than an on-device collective here.

Attention computes S^T = K_blk^T Q_blk per granted block with the 4 GQA
heads batched side-by-side in the moving operand (N=512 matmuls — the PE
sequencer is the co-bottleneck at N=128): exp(S^T) lands in SBUF as P^T
tiles for the P^T @ V accumulation, softmax column sums come from a DVE
free-axis reduce over the per-block tiles plus a single all-ones stationary
matmul (which broadcasts them across partitions for free), and the 1/sum
normalization is deferred one block so the PE never waits on the softmax
chain. V is projected directly in the natural [s, d] PV layout.

Everything on device runs in bf16 with fp32 PSUM accumulation.
"""

import sys

sys.path.insert(0, "/opt/trn_rl_repo")

import math
from contextlib import ExitStack

import ml_dtypes
import numpy as np

import concourse.bass as bass
import concourse.tile as tile
from concourse import bacc, mybir
from concourse.masks import make_lower_triangular

BF16 = mybir.dt.bfloat16
F32 = mybir.dt.float32
NPBF = ml_dtypes.bfloat16

N_CORES = 8
S = 4096
HID = 4096
NH, NKV, D = 32, 8, 128
QH = NH // N_CORES          # 4 q heads per core
BLK = 128
NB = S // BLK               # 32 blocks
LOCAL_NB = 8
SCHUNK = 512                # s-columns processed per phase-1 step
NSC = S // SCHUNK           # 8
HT = HID // 128             # 32 contraction tiles
MASK_VAL = -30000.0
THETA = 10000.0


def _rope_into(nc, pool, dst, ps, cos_c, sin_c, width):
    """dst(bf16) = ps * cos_c + swap_halves(ps) * sin_c  (sin_c sign-baked).

    ps is a [128, width] fp32 PSUM tile holding a projection output d-block;
    partition p is feature dim d. swap_halves pairs d <-> d+64.
    dst may be a 3-D AP; t0/t1 are viewed with the same shape for the add.
    """
    t0 = pool.tile([128, SCHUNK], F32, tag="rope_t0", name="rope_t0")
    t1 = pool.tile([128, SCHUNK], F32, tag="rope_t1", name="rope_t1")
    nc.vector.tensor_mul(t0[:, :width], ps[:, :width], cos_c[:, :width])
    nc.vector.tensor_mul(t1[0:64, :width], ps[64:128, :width], sin_c[0:64, :width])
    nc.vector.tensor_mul(t1[64:128, :width], ps[0:64, :width], sin_c[64:128, :width])
    if len(dst.shape) == 3:
        b = dst.shape[1]
        qw = dst.shape[2]
        nc.vector.tensor_add(
            dst,
            t0[:, :width].rearrange("p (b q) -> p b q", b=b, q=qw),
            t1[:, :width].rearrange("p (b q) -> p b q", b=b, q=qw),
        )
    else:
        nc.vector.tensor_add(dst, t0[:, :width], t1[:, :width])


def _emit_body(nc, tc, persist, aps):
    hsT, wq, wk, wv, wo, cos2, sin2, out_p = aps

    # Head-batched layouts: block i occupies cols [i*512, (i+1)*512), with the
    # 4 q-heads side by side (head hq at [i*512 + hq*128, ...)). This makes the
    # attention matmuls N=512 (one per k-block, covering all 4 GQA heads that
    # share this core's kv head) instead of N=128 per (head, block) — 4x fewer
    # PE instructions, which matters because the PE NX sequencer is the
    # co-bottleneck at N=128.
    qT4 = persist.tile([128, NB * 512], BF16, name="qT4")     # [d | (blk, hq, q)]
    kT = persist.tile([128, S], BF16, name="kT")              # [d | s]
    vN = persist.tile([128, NB * 128], BF16, name="vN")       # [s_in_blk | (blk, d)]
    attnT4 = persist.tile([128, NB * 512], BF16, name="attnT4")  # like qT4
    wq_sb = persist.tile([128, HT * QH * 128], BF16, name="wq_sb")
    wk_sb = persist.tile([128, HT * 128], BF16, name="wk_sb")
    wv_sb = persist.tile([128, HT * 128], BF16, name="wv_sb")
    wo_sb = persist.tile([128, QH * HID], BF16, name="wo_sb")
    ones = persist.tile([128, 128], BF16, name="ones")
    tri4 = persist.tile([128, 512], F32, name="tri4")   # causal mask x4 heads

    nc.vector.memset(ones, 1.0)
    make_lower_triangular(nc, tri4[:, 0:128], val=MASK_VAL, diag=False)
    for r in range(1, 4):
        nc.vector.tensor_copy(tri4[:, r * 128 : (r + 1) * 128], tri4[:, 0:128])

    GRP = 8                      # htiles per grouped DMA
    WGRP = GRP * 128

    def _load_w_group(g):
        # one 3D-AP DMA per weight for 8 htiles: [ht*128+p, c] -> [p, ht*C+c]
        nc.sync.dma_start(
            out=wq_sb[:, g * GRP * 512 : (g + 1) * GRP * 512].rearrange(
                "p (t c) -> p t c", c=512
            ),
            in_=wq[g * WGRP : (g + 1) * WGRP, :].rearrange("(t p) c -> p t c", p=128),
        )
        nc.sync.dma_start(
            out=wk_sb[:, g * GRP * 128 : (g + 1) * GRP * 128].rearrange(
                "p (t c) -> p t c", c=128
            ),
            in_=wk[g * WGRP : (g + 1) * WGRP, :].rearrange("(t p) c -> p t c", p=128),
        )
        nc.sync.dma_start(
            out=wv_sb[:, g * GRP * 128 : (g + 1) * GRP * 128].rearrange(
                "p (t c) -> p t c", c=128
            ),
            in_=wv[g * WGRP : (g + 1) * WGRP, :].rearrange("(t p) c -> p t c", p=128),
        )

    # ---------------- phase 1: projections + RoPE ----------------
    ph1 = ExitStack()
    hs_pool = ph1.enter_context(tc.tile_pool(name="hs_pool", bufs=4))
    tab_pool = ph1.enter_context(tc.tile_pool(name="tab_pool", bufs=1))
    rope_pool = ph1.enter_context(tc.tile_pool(name="rope_pool", bufs=1))
    ps_p = ph1.enter_context(tc.tile_pool(name="ps_p", bufs=6, space="PSUM"))
    ps_v = ph1.enter_context(tc.tile_pool(name="ps_v", bufs=2, space="PSUM"))

    def _q_dst(sc, db):
        # strided view of qT4 for this chunk+head: [p, 4 blocks, 128 q]
        return qT4[:, sc * 2048 : (sc + 1) * 2048].rearrange(
            "p (b q) -> p b q", q=512
        )[:, :, db * 128 : (db + 1) * 128]

    def _v_pass(sc, _hs):
        # v projection in natural [s, d] layout (stationary = hs slice):
        # the PV matmul needs s on partitions, and this form is proven to
        # run at model rate on hardware.
        for sb in range(SCHUNK // 128):
            g = sc * (SCHUNK // 128) + sb
            psv = ps_v.tile([128, 128], F32, tag="ps_v", name="ps_vp")
            for ht in range(HT):
                nc.tensor.matmul(
                    psv,
                    lhsT=_hs(ht)[:, sb * 128 : (sb + 1) * 128],
                    rhs=wv_sb[:, ht * 128 : (ht + 1) * 128],
                    start=(ht == 0),
                    stop=(ht == HT - 1),
                )
            nc.scalar.copy(out=vN[:, g * 128 : (g + 1) * 128], in_=psv)

    for sc in range(NSC):
        scol = slice(sc * SCHUNK, (sc + 1) * SCHUNK)
        cos_c = tab_pool.tile([128, SCHUNK], F32, tag="cos_c", name="cos_c")
        sin_c = tab_pool.tile([128, SCHUNK], F32, tag="sin_c", name="sin_c")
        hs_grps = []
        for g in range(HT // GRP):
            if sc == 0:
                # interleave weight group loads with the first hs chunk so
                # the first matmuls aren't stuck behind the weight prefetch
                _load_w_group(g)
            hg = hs_pool.tile([128, GRP * SCHUNK], BF16, tag="hs",
                              name=f"hs_{sc}_{g}")
            nc.sync.dma_start(
                out=hg.rearrange("p (t c) -> p t c", c=SCHUNK),
                in_=hsT[g * WGRP : (g + 1) * WGRP, scol].rearrange(
                    "(t p) c -> p t c", p=128
                ),
            )
            hs_grps.append(hg)

        def _hs(ht):
            return hs_grps[ht // GRP][:, (ht % GRP) * SCHUNK : (ht % GRP + 1) * SCHUNK]
        # tables are consumed by RoPE at chunk end; keep them out of the
        # DMA queue's critical head during the compute-feeding loads
        nc.sync.dma_start(out=cos_c, in_=cos2[:, scol])
        nc.sync.dma_start(out=sin_c, in_=sin2[:, scol])
        if sc == 0:
            # wo is only read in phase 2; load it behind everything else
            nc.sync.dma_start(
                out=wo_sb.rearrange("p (t c) -> p t c", c=HID),
                in_=wo.rearrange("(t p) c -> p t c", p=128),
            )
        # q (4 head d-blocks) + k + v projections, all in transposed layout
        # [d, s] with N=512 matmuls. For the first chunk go ht-outer with all
        # 6 psums live so PE has ~1.3us of work per arriving hs tile
        # (DMA-matched at startup); later chunks are fully prefetched and use
        # the output-outer order (1 psum bank at a time).
        if sc == 0:
            ps5 = [
                ps_p.tile([128, SCHUNK], F32, tag="pp", name=f"ps5_{r}")
                for r in range(5)
            ]
            for ht in range(HT):
                for db in range(QH):
                    nc.tensor.matmul(
                        ps5[db],
                        lhsT=wq_sb[:, ht * 512 + db * 128 : ht * 512 + (db + 1) * 128],
                        rhs=_hs(ht),
                        start=(ht == 0),
                        stop=(ht == HT - 1),
                    )
                nc.tensor.matmul(
                    ps5[4],
                    lhsT=wk_sb[:, ht * 128 : (ht + 1) * 128],
                    rhs=_hs(ht),
                    start=(ht == 0),
                    stop=(ht == HT - 1),
                )
            for db in range(QH):
                _rope_into(nc, rope_pool, _q_dst(sc, db), ps5[db],
                           cos_c, sin_c, SCHUNK)
            _rope_into(nc, rope_pool, kT[:, scol], ps5[4], cos_c, sin_c, SCHUNK)
            _v_pass(sc, _hs)
        else:
            # v first, then q, then k last — so the hs groups' last reader is
            # the k-pass's early htiles, releasing buffers for the next
            # chunk's prefetch as early as possible.
            _v_pass(sc, _hs)
            for db in range(QH):
                ps = ps_p.tile([128, SCHUNK], F32, tag="pp", name="ps_qp")
                for ht in range(HT):
                    nc.tensor.matmul(
                        ps,
                        lhsT=wq_sb[:, ht * 512 + db * 128 : ht * 512 + (db + 1) * 128],
                        rhs=_hs(ht),
                        start=(ht == 0),
                        stop=(ht == HT - 1),
                    )
                _rope_into(nc, rope_pool, _q_dst(sc, db), ps,
                           cos_c, sin_c, SCHUNK)
            psk = ps_p.tile([128, SCHUNK], F32, tag="pp", name="ps_kp")
            for ht in range(HT):
                nc.tensor.matmul(
                    psk,
                    lhsT=wk_sb[:, ht * 128 : (ht + 1) * 128],
                    rhs=_hs(ht),
                    start=(ht == 0),
                    stop=(ht == HT - 1),
                )
            _rope_into(nc, rope_pool, kT[:, scol], psk, cos_c, sin_c, SCHUNK)
    ph1.close()

    # -------- phase 2+3 merged: attention with o_proj software-pipelined --------
    # Per block i (all 4 heads at once, N=512):
    #   S^T_j = kT_j^T @ qT4_i per granted block j  ->  exp on ACT  ->
    #   PV accumulates over j; softmax sums via a DVE free-axis reduce over the
    #   j tiles + ONE all-ones matmul (partition-broadcast); normalize is
    #   deferred one iteration, o_proj runs two blocks behind as PE filler
    #   while ACT works through the exp chain.
    ph2 = ExitStack()
    pt_pool = ph2.enter_context(tc.tile_pool(name="pt_pool", bufs=3))
    red_pool = ph2.enter_context(tc.tile_pool(name="red_pool", bufs=2))
    rb_pool = ph2.enter_context(tc.tile_pool(name="rb_pool", bufs=2))
    ob_pool = ph2.enter_context(tc.tile_pool(name="ob_pool", bufs=4))
    ps_S = ph2.enter_context(tc.tile_pool(name="ps_S", bufs=3, space="PSUM"))
    ps_O = ph2.enter_context(tc.tile_pool(name="ps_O", bufs=2, space="PSUM"))
    ps_sum = ph2.enter_context(tc.tile_pool(name="ps_sum", bufs=1, space="PSUM"))
    ps_o3 = ph2.enter_context(tc.tile_pool(name="ps_o3", bufs=2, space="PSUM"))

    def _og(sb, ct):
        # o_proj partial for s-block sb, output cols [ct*512, (ct+1)*512)
        ps = ps_o3.tile([128, 512], F32, tag="o3", name="o3")
        for ht in range(QH):
            nc.tensor.matmul(
                ps,
                lhsT=attnT4[:, (sb * 4 + ht) * 128 : (sb * 4 + ht + 1) * 128],
                rhs=wo_sb[:, ht * HID + ct * 512 : ht * HID + (ct + 1) * 512],
                start=(ht == 0),
                stop=(ht == QH - 1),
            )
        ob = ob_pool.tile([128, 512], BF16, tag="ob", name="ob")
        # split the PSUM->SBUF drain 6:2 toward ACT: on HW the DVE's f32 adds
        # run ~20% over model, making DVE co-critical with the PE, while ACT
        # (measured at model rate) has ~2.7us/iter of slack
        if ct % 4 != 3:
            nc.scalar.copy(out=ob, in_=ps)
        else:
            nc.vector.tensor_copy(ob, ps)
        nc.sync.dma_start(
            out=out_p[sb * 128 : (sb + 1) * 128, ct * 512 : (ct + 1) * 512],
            in_=ob,
        )

    def _finish(pend):
        # ones-matmul broadcast of the j-reduced sums, then 1/sum and the
        # deferred normalize of block ip's attention output.
        red_bf, O_prev, ip = pend
        sum_ps = ps_sum.tile([128, 512], F32, tag="sum", name="sum_ps")
        nc.tensor.matmul(sum_ps, lhsT=ones, rhs=red_bf, start=True, stop=True)
        rb = rb_pool.tile([128, 512], F32, tag="rb", name="rb")
        nc.vector.reciprocal_approx_fast(rb, sum_ps)
        nc.vector.tensor_mul(
            attnT4[:, ip * 512 : (ip + 1) * 512], O_prev, rb
        )

    pending = None
    for i in range(NB):
        L = min(i, LOCAL_NB)       # number of local blocks
        js = i - L + 1             # first local block (>= 1 when L > 0)
        blocks = list(range(js, i + 1)) if i >= 1 else []
        blocks.append(0)           # sink block last
        nblk = len(blocks)
        diag_bi = L - 1 if i >= 1 else 0
        qs = qT4[:, i * 512 : (i + 1) * 512]
        PT_all = pt_pool.tile([128, 9 * 512], BF16, tag="PT", name="PT_all")
        acc = red_pool.tile([128, 512], F32, tag="acc", name="acc") \
            if nblk > 1 else None

        def _st(bi, j):
            sp = ps_S.tile([128, 512], F32, tag="S", name="S_ps")
            nc.tensor.matmul(sp, lhsT=kT[:, j * 128 : (j + 1) * 128], rhs=qs,
                             start=True, stop=True)
            if bi == diag_bi:
                nc.vector.tensor_add(sp, sp, tri4)
            nc.scalar.activation(
                out=PT_all[:, bi * 512 : (bi + 1) * 512], in_=sp,
                func=mybir.ActivationFunctionType.Exp,
            )
            # fold the fresh exp tile into the f32 softmax-sum accumulator
            # right away: the adds ride along behind ACT's exp stream, so the
            # total is ready ~one add after the last exp
            if bi == 1:
                nc.vector.tensor_add(acc, PT_all[:, 0:512], PT_all[:, 512:1024])
            elif bi >= 2:
                nc.vector.tensor_add(
                    acc, acc, PT_all[:, bi * 512 : (bi + 1) * 512]
                )

        for bi in range(min(3, nblk)):
            _st(bi, blocks[bi])
        if pending is not None:
            _finish(pending)
        # interleave the remaining S^T with o_proj filler for block i-2
        ogs = list(range(8)) if i >= 2 else []
        for bi in range(3, nblk):
            if ogs:
                _og(i - 2, ogs.pop(0))
            _st(bi, blocks[bi])
        for ct in ogs:
            _og(i - 2, ct)
        # P^T @ V accumulated over granted blocks
        O_ps = ps_O.tile([128, 512], F32, tag="O", name="O_ps")
        for bi, j in enumerate(blocks):
            nc.tensor.matmul(
                O_ps, lhsT=vN[:, j * 128 : (j + 1) * 128],
                rhs=PT_all[:, bi * 512 : (bi + 1) * 512],
                start=(bi == 0), stop=(bi == nblk - 1),
            )
        # softmax sums: cast the f32 accumulator for the all-ones matmul
        if nblk > 1:
            red_bf = red_pool.tile([128, 512], BF16, tag="redb", name="redb")
            nc.scalar.copy(out=red_bf, in_=acc)
        else:
            red_bf = PT_all[:, 0:512]
        pending = (red_bf, O_ps, i)

    _finish(pending)
    for sb in (NB - 2, NB - 1):
        for ct in range(8):
            _og(sb, ct)
    ph2.close()


def build_kernel(nc, reps=1):
    hsT = nc.dram_tensor("hsT", [HID, S], BF16, kind="ExternalInput").ap()
    wq = nc.dram_tensor("wq", [HID, QH * D], BF16, kind="ExternalInput").ap()
    wk = nc.dram_tensor("wk", [HID, D], BF16, kind="ExternalInput").ap()
    wv = nc.dram_tensor("wv", [HID, D], BF16, kind="ExternalInput").ap()
    wo = nc.dram_tensor("wo", [QH * D, HID], BF16, kind="ExternalInput").ap()
    cos2 = nc.dram_tensor("cos2", [128, S], F32, kind="ExternalInput").ap()
    sin2 = nc.dram_tensor("sin2", [128, S], F32, kind="ExternalInput").ap()
    out_p = nc.dram_tensor("out_p", [S, HID], BF16, kind="ExternalOutput").ap()
    aps = (hsT, wq, wk, wv, wo, cos2, sin2, out_p)

    with tile.TileContext(nc) as tc:
        with tc.tile_pool(name="persist", bufs=1) as persist:
            for _rep in range(reps):
                _emit_body(nc, tc, persist, aps)
    return nc


_NC = {}


def _get_nc(reps=1):
    if reps not in _NC:
        nc = bacc.Bacc(
            "TRN2", target_bir_lowering=False, debug=False, num_devices=N_CORES
        )
        build_kernel(nc, reps=reps)
        nc.compile()
        _NC[reps] = nc
    return _NC[reps]


def make_exec_fn(nc, n_cores=N_CORES):
    """Build a reusable sharded executor for a compiled Bass module.

    Mirrors bass2jax.run_bass_via_pjrt's multi-core branch, but without
    donation so the zero output buffers can stay device-resident across
    repeated calls (for benchmarking).
    """
    import jax
    from jax.sharding import Mesh, NamedSharding, PartitionSpec
    from jax.experimental.shard_map import shard_map

    from concourse import bass2jax

    bass2jax.install_neuronx_cc_hook()

    partition_name = nc.partition_id_tensor.name if nc.partition_id_tensor else None
    in_names, out_names, out_avals, zero_outs = [], [], [], []
    for alloc in nc.m.functions[0].allocations:
        if not isinstance(alloc, mybir.MemoryLocationSet):
            continue
        name = alloc.memorylocations[0].name
        if alloc.kind == "ExternalInput":
            if name != partition_name:
                in_names.append(name)
        elif alloc.kind == "ExternalOutput":
            out_names.append(name)
            shape = tuple(alloc.tensor_shape)
            dtype = mybir.dt.np(alloc.dtype)
            out_avals.append(jax.core.ShapedArray(shape, dtype))
            zero_outs.append(np.zeros(shape, dtype))
    all_in_names = list(in_names) + list(out_names)
    if partition_name is not None:
        all_in_names.append(partition_name)
    all_in_names = tuple(all_in_names)

    def _body(*args):
        operands = list(args)
        if partition_name is not None:
            operands.append(bass2jax.partition_id_tensor())
        outs = bass2jax._bass_exec_p.bind(
            *operands,
            out_avals=tuple(out_avals),
            in_names=all_in_names,
            out_names=tuple(out_names),
            lowering_input_output_aliases=(),
            sim_require_finite=True,
            sim_require_nnan=True,
            nc=nc,
        )
        return tuple(outs)

    devices = jax.devices()[:n_cores]
    mesh = Mesh(np.asarray(devices), ("core",))
    spec = PartitionSpec("core")
    in_specs = (spec,) * (len(in_names) + len(out_names))
    out_specs = (spec,) * len(out_names)
    fn = jax.jit(
        shard_map(
            _body, mesh=mesh, in_specs=in_specs, out_specs=out_specs, check_rep=False
        ),
        keep_unused=True,
    )
    return fn, in_names, out_names, zero_outs, NamedSharding(mesh, spec)


_EXEC = None


def _get_exec():
    global _EXEC
    if _EXEC is None:
        _EXEC = make_exec_fn(_get_nc())
    return _EXEC


def _concat_args(in_maps, in_names, zero_outs):
    concat_in = [
        np.concatenate([np.asarray(in_maps[c][nm]) for c in range(N_CORES)], axis=0)
        for nm in in_names
    ]
    concat_zeros = [
        np.zeros((N_CORES * z.shape[0], *z.shape[1:]), z.dtype) for z in zero_outs
    ]
    return concat_in + concat_zeros


def _host_inputs(hidden_states, wq, wk, wv, wo):
    hs = np.asarray(hidden_states, np.float32).reshape(S, HID)
    hsT = np.ascontiguousarray(hs.T).astype(NPBF)

    scale = 1.0 / math.sqrt(D)
    inv_freq = 1.0 / (THETA ** (np.arange(0, D, 2, dtype=np.float32) / D))
    t = np.arange(S, dtype=np.float32)
    freqs = np.outer(t, inv_freq)                      # [S, 64]
    cosT = np.cos(freqs).T.astype(np.float32)          # [64, S]
    sinT = np.sin(freqs).T.astype(np.float32)
    cos2 = np.ascontiguousarray(np.concatenate([cosT, cosT], 0))   # [128, S]
    sin2 = np.ascontiguousarray(np.concatenate([-sinT, sinT], 0))  # [128, S]

    wq = np.asarray(wq, np.float32) * scale
    in_maps = []
    for c in range(N_CORES):
        in_maps.append(
            {
                "hsT": hsT,
                "wq": np.ascontiguousarray(wq[:, c * 512 : (c + 1) * 512]).astype(NPBF),
                "wk": np.ascontiguousarray(
                    np.asarray(wk, np.float32)[:, c * 128 : (c + 1) * 128]
                ).astype(NPBF),
                "wv": np.ascontiguousarray(
                    np.asarray(wv, np.float32)[:, c * 128 : (c + 1) * 128]
                ).astype(NPBF),
                "wo": np.ascontiguousarray(
                    np.asarray(wo, np.float32)[c * 512 : (c + 1) * 512, :]
                ).astype(NPBF),
                "cos2": cos2,
                "sin2": sin2,
            }
        )
    return in_maps


def _reduce_out(out_concat):
    acc = (
        np.asarray(out_concat)
        .reshape(N_CORES, S, HID)
        .astype(np.float32)
        .sum(axis=0)
    )
    return np.ascontiguousarray(acc).reshape(1, S, HID)


def run(hidden_states, wq, wk, wv, wo):
    """Returns full fp32 output [1, S, HID]."""
    import jax

    fn, in_names, out_names, zero_outs, sh = _get_exec()
    in_maps = _host_inputs(hidden_states, wq, wk, wv, wo)
    args = _concat_args(in_maps, in_names, zero_outs)
    outs = jax.block_until_ready(fn(*args))
    return _reduce_out(outs[0])


def bench(hidden_states, wq, wk, wv, wo, iters=10):
    """Repeated device-resident executions; returns (out, per-iter seconds)."""
    import time

    import jax

    fn, in_names, out_names, zero_outs, sh = _get_exec()
    in_maps = _host_inputs(hidden_states, wq, wk, wv, wo)
    args = _concat_args(in_maps, in_names, zero_outs)
    dev_args = jax.block_until_ready([jax.device_put(a, sh) for a in args])
    outs = jax.block_until_ready(fn(*dev_args))  # warm-up + compile
    times = []
    for _ in range(iters):
        t0 = time.perf_counter()
        o = fn(*dev_args)
        jax.block_until_ready(o)
        times.append(time.perf_counter() - t0)
    # async-queued: submit all, block once — measures pipelined dispatch
    for n in (1, iters):
        t0 = time.perf_counter()
        os_ = [fn(*dev_args) for _ in range(n)]
        jax.block_until_ready(os_)
        times.append((time.perf_counter() - t0) / n)
    return _reduce_out(outs[0]), times


def kernel(hidden_states, wq, wk, wv, wo):
    return run(hidden_states, wq, wk, wv, wo)

